# revision 12
# baseline (speedup 1.0000x reference)
"""Multi-head attention (B=8, N=1024, C=768, H=12, D=64) on 8 TRN2 NeuronCores.

Sharding: pure data parallelism — one batch element per core, no collectives.

Per-core dataflow (matmuls bf16, accumulation fp32 in PSUM):
  host prep: x[b].T, per-section qkv weights and proj weight transposed to
  [in, out], all bf16; proj bias fp32 [128, 6]; ones2 bf16 [2, 128]
  (indicator rows to broadcast softmax reciprocals across partitions).

  Every matmul window is paired with a sibling sharing the same stationary
  operand (the two 512-wide n-chunks), so each weight load covers two
  512-column windows.

  C: v natural [N, 12*128] (64 data + ones col + zero pad -> M=128 AV)
  B: qT, kT [C, N] (head-dim on partitions)
  D: per head pair: scores S^T[m, n] as K=64 row-group matmuls, exp on
     ScalarE (1024 wide, scale fused) into bf16 P tiles; AV with M=128
     (row 64 = softmax denominator); raw out^T to SBUF; denominator rows
     collected into a [96, 128] tile; every odd hp, a 32-row reciprocal
     chunk + broadcast matmuls + DVE multiplies normalize finished blocks.
  E: yT[o, n] = pwT.T @ outT with bias added on DVE.
Host transposes yT back to [N, C].
"""

import numpy as np

B, N, C, H, D = 8, 1024, 768, 12, 64
SCALE = D ** -0.5
NCORES = 8

CT = C // 128   # 6  c-tiles
HP = H // 2     # 6  head pairs (2 heads of 64 share a 128-partition tile)
NT = N // 512   # 2  n-chunks of 512
MT = N // 128   # 8  m-tiles (keys)
VW = 128        # v cols per head (64 data + 1 ones + 63 pad; 128 enables FWL)

_CACHE = {}


def _build_nc():
    import concourse.bass as bass
    import concourse.mybir as mybir
    import concourse.tile as tile
    from concourse import bacc

    f32 = mybir.dt.float32
    bf16 = mybir.dt.bfloat16
    AF = mybir.ActivationFunctionType

    nc = bacc.Bacc(
        "TRN2",
        target_bir_lowering=False,
        debug=False,
        enable_asserts=True,
        num_devices=NCORES,
    )

    xT_d = nc.dram_tensor("xT", [C, N], bf16, kind="ExternalInput").ap()
    wq_d = nc.dram_tensor("wqT", [C, C], bf16, kind="ExternalInput").ap()
    wk_d = nc.dram_tensor("wkT", [C, C], bf16, kind="ExternalInput").ap()
    wv_d = nc.dram_tensor("wvT", [C, C], bf16, kind="ExternalInput").ap()
    pw_d = nc.dram_tensor("pwT", [C, C], bf16, kind="ExternalInput").ap()
    ones_d = nc.dram_tensor("ones2", [2, 128], bf16, kind="ExternalInput").ap()
    pb_d = nc.dram_tensor("pb", [128, CT], f32, kind="ExternalInput").ap()
    out_d = nc.dram_tensor("out", [C, N], f32, kind="ExternalOutput").ap()

    with tile.TileContext(nc) as tc:
        data = tc.alloc_tile_pool(name="data", bufs=1)
        ptp = tc.alloc_tile_pool(name="ptp", bufs=1)
        small = tc.alloc_tile_pool(name="small", bufs=2)
        psp = tc.alloc_tile_pool(name="psp", bufs=1, space="PSUM")

        pb_sb = data.tile([128, CT], f32)
        nc.sync.dma_start(pb_sb[:], pb_d[:])
        ones2 = data.tile([2, 128], bf16)
        nc.sync.dma_start(ones2[:], ones_d[:])

        xTs = data.tile([128, CT * N], bf16)
        wvs = data.tile([128, CT * C], bf16)
        wqs = data.tile([128, CT * C], bf16)
        wks = data.tile([128, CT * C], bf16)
        pws = data.tile([128, CT * C], bf16)
        for ci in range(CT):
            nc.sync.dma_start(xTs[:, ci * N:(ci + 1) * N],
                              xT_d[ci * 128:(ci + 1) * 128, :])
            nc.sync.dma_start(wvs[:, ci * C:(ci + 1) * C],
                              wv_d[ci * 128:(ci + 1) * 128, :])
            nc.sync.dma_start(wqs[:, ci * C:(ci + 1) * C],
                              wq_d[ci * 128:(ci + 1) * 128, :])
            nc.sync.dma_start(wks[:, ci * C:(ci + 1) * C],
                              wk_d[ci * 128:(ci + 1) * 128, :])
            nc.sync.dma_start(pws[:, ci * C:(ci + 1) * C],
                              pw_d[ci * 128:(ci + 1) * 128, :])

        qT = data.tile([128, HP * N], bf16)
        kT = data.tile([128, HP * N], bf16)
        va = data.tile([128, MT * H * VW], bf16)
        oT_raw = data.tile([128, HP * N], bf16)
        oT = data.tile([128, HP * N], bf16)
        den = data.tile([96, 128], f32)
        recip_all = data.tile([96, 128], bf16)

        # zero-fill + ones columns of v (softmax denominator trick)
        nc.gpsimd.memset(va[:], 0.0)
        v3 = va[:].rearrange("p (x e) -> p x e", e=VW)
        nc.gpsimd.memset(v3[:, :, 64:65], 1.0)

        # ---- C: v natural layout [tokens, head*VW] -----------------------
        for mt in range(MT):
            ps = psp.tile([128, 1024], f32, tag="st", bufs=2, name="ps_v")
            for ci in range(CT):
                lhs = xTs[:, ci * N + mt * 128: ci * N + mt * 128 + 128]
                nc.tensor.matmul(ps[:, 0:512], lhs,
                                 wvs[:, ci * C: ci * C + 512],
                                 start=(ci == 0), stop=(ci == CT - 1))
                nc.tensor.matmul(ps[:, 512:768], lhs,
                                 wvs[:, ci * C + 512: ci * C + 768],
                                 start=(ci == 0), stop=(ci == CT - 1))
            dst3 = va[:, mt * H * VW:(mt + 1) * H * VW].rearrange(
                "p (h e) -> p h e", e=VW)[:, :, 0:64]
            src3 = ps[:, 0:768].rearrange("p (h d) -> p h d", d=64)
            nc.vector.tensor_copy(dst3, src3)

        # ---- B: qT / kT (transposed projections) -------------------------
        for hp in range(HP):
            for dst, w in ((qT, wqs), (kT, wks)):
                ps = psp.tile([128, 1024], f32, tag="st", bufs=2, name="ps_qk")
                for ci in range(CT):
                    lhs = w[:, ci * C + hp * 128: ci * C + hp * 128 + 128]
                    nc.tensor.matmul(ps[:, 0:512], lhs,
                                     xTs[:, ci * N: ci * N + 512],
                                     start=(ci == 0), stop=(ci == CT - 1))
                    nc.tensor.matmul(ps[:, 512:1024], lhs,
                                     xTs[:, ci * N + 512: ci * N + 1024],
                                     start=(ci == 0), stop=(ci == CT - 1))
                nc.vector.tensor_copy(dst[:, hp * N: hp * N + 1024], ps[:])

        # ---- softmax division (chunked so E can start early) -------------
        def do_division(chunk):
            rows = slice(32 * chunk, 32 * chunk + 32)
            with nc.allow_low_precision(reason="softmax denom in bf16"):
                nc.vector.reciprocal(recip_all[rows, :], den[rows, :])
            for j in range(4 * chunk, 4 * chunk + 4):
                hp_, nn_ = divmod(j, NT)
                recip2 = small.tile([2, 512], bf16, tag="recip2", name="recip2")
                for hi in range(2):
                    idx = 2 * j + hi
                    nc.sync.dma_start(recip2[hi:hi + 1, :],
                                      recip_all[4 * idx:4 * idx + 4, :])
                bc = psp.tile([128, 512], f32, tag="out", bufs=2, name="bc")
                nc.tensor.matmul(bc[:], ones2[:], recip2[:], start=True,
                                 stop=True)
                blk = slice(hp_ * N + nn_ * 512, hp_ * N + nn_ * 512 + 512)
                nc.vector.tensor_mul(oT[:, blk], oT_raw[:, blk], bc[:])

        # ---- D: attention per head pair ----------------------------------
        for hp in range(HP):
            pt = ptp.tile([128, 2 * MT * 1024], bf16, tag="pt", name="pt")
            for mt in range(MT):
                for hi in range(2):
                    lo = 64 * hi
                    st = psp.tile([128, 1024], f32, tag="st", bufs=2, name="st")
                    lhs = kT[lo:lo + 64,
                             hp * N + mt * 128: hp * N + mt * 128 + 128]
                    nc.tensor.matmul(st[:, 0:512], lhs,
                                     qT[lo:lo + 64, hp * N: hp * N + 512])
                    nc.tensor.matmul(st[:, 512:1024], lhs,
                                     qT[lo:lo + 64,
                                        hp * N + 512: hp * N + 1024])
                    nc.scalar.activation(
                        pt[:, hi * MT * 1024 + mt * 1024:
                           hi * MT * 1024 + mt * 1024 + 1024],
                        st[:], AF.Exp, scale=SCALE)
            for hi in range(2):
                h = 2 * hp + hi
                lo = 64 * hi
                av0 = psp.tile([128, 512], f32, tag="acc", bufs=2, name="av0")
                av1 = psp.tile([128, 512], f32, tag="acc", bufs=2, name="av1")
                avs = [av0, av1]
                for mt in range(MT):
                    lhs = va[:, mt * H * VW + h * VW: mt * H * VW + h * VW + VW]
                    for nn in range(NT):
                        nc.tensor.matmul(
                            avs[nn][:], lhs,
                            pt[:, hi * MT * 1024 + mt * 1024 + nn * 512:
                               hi * MT * 1024 + mt * 1024 + nn * 512 + 512],
                            start=(mt == 0), stop=(mt == MT - 1),
                        )
                for nn in range(NT):
                    av = avs[nn]
                    nc.vector.tensor_copy(
                        oT_raw[lo:lo + 64,
                               hp * N + nn * 512: hp * N + nn * 512 + 512],
                        av[0:64, :])
                    idx = (hp * NT + nn) * 2 + hi
                    dstage = small.tile([65, 512], f32, tag="dstage",
                                        name="dstage")
                    nc.vector.tensor_copy(dstage[64:65, :], av[64:65, :])
                    nc.sync.dma_start(den[4 * idx:4 * idx + 4, :],
                                      dstage[64:65, :])
            if hp % 2 == 1:
                do_division(hp // 2)

        # ---- E: output projection ----------------------------------------
        for ot in range(CT):
            yp = psp.tile([128, 1024], f32, tag="st", bufs=2, name="yp")
            for ci in range(CT):
                lhs = pws[:, ci * C + ot * 128: ci * C + ot * 128 + 128]
                nc.tensor.matmul(yp[:, 0:512], lhs,
                                 oT[:, ci * N: ci * N + 512],
                                 start=(ci == 0), stop=(ci == CT - 1))
                nc.tensor.matmul(yp[:, 512:1024], lhs,
                                 oT[:, ci * N + 512: ci * N + 1024],
                                 start=(ci == 0), stop=(ci == CT - 1))
            ys = small.tile([128, 1024], f32, tag="ys", name="ys")
            nc.vector.tensor_scalar_add(ys[:], yp[:], pb_sb[:, ot:ot + 1])
            nc.sync.dma_start(out_d[ot * 128:(ot + 1) * 128, :], ys[:])

        psp.release()
        small.release()
        ptp.release()
        data.release()

    nc.compile()
    return nc


def _get_nc():
    if "nc" not in _CACHE:
        _CACHE["nc"] = _build_nc()
    return _CACHE["nc"]


def _prep_in_maps(x, qkv_w, proj_w, proj_b):
    import ml_dtypes

    bf16 = ml_dtypes.bfloat16
    x = np.asarray(x, dtype=np.float32)
    qkv_w = np.asarray(qkv_w, dtype=np.float32)
    proj_w = np.asarray(proj_w, dtype=np.float32)
    proj_b = np.asarray(proj_b, dtype=np.float32)

    wqT = np.ascontiguousarray(qkv_w[0:C].T).astype(bf16)     # [C(in), C(out)]
    wkT = np.ascontiguousarray(qkv_w[C:2 * C].T).astype(bf16)
    wvT = np.ascontiguousarray(qkv_w[2 * C:3 * C].T).astype(bf16)
    pwT = np.ascontiguousarray(proj_w.T).astype(bf16)
    pb = np.ascontiguousarray(proj_b.reshape(CT, 128).T)      # [128, CT] f32
    ones2 = np.zeros((2, 128), dtype=np.float32)
    ones2[0, 0:64] = 1.0
    ones2[1, 64:128] = 1.0
    ones2 = ones2.astype(bf16)

    in_maps = []
    for b in range(B):
        in_maps.append({
            "xT": np.ascontiguousarray(x[b].T).astype(bf16),
            "wqT": wqT, "wkT": wkT, "wvT": wvT, "pwT": pwT, "pb": pb,
            "ones2": ones2,
        })
    return in_maps


def _run(in_maps, **kwargs):
    from concourse.bass_utils import run_bass_kernel_spmd

    return run_bass_kernel_spmd(_get_nc(), in_maps,
                                core_ids=list(range(NCORES)), **kwargs)


def _gather(res):
    out = np.stack([res.results[b]["out"].T for b in range(B)], axis=0)
    return np.ascontiguousarray(out.astype(np.float32))


def kernel(x, qkv_w, proj_w, proj_b):
    return _gather(_run(_prep_in_maps(x, qkv_w, proj_w, proj_b)))


# revision 13
# speedup vs baseline: 1.1860x; 1.1860x over previous
"""Multi-head attention (B=8, N=1024, C=768, H=12, D=64) on 8 TRN2 NeuronCores.

Sharding: pure data parallelism — one batch element per core, no collectives.

Per-core dataflow (matmuls bf16, accumulation fp32 in PSUM):
  host prep: x[b].T, per-section qkv weights and proj weight transposed to
  [in, out], all bf16; proj bias fp32 [128, 6]; ones2 bf16 [2, 128]
  (indicator rows to broadcast softmax reciprocals across partitions).

  B: qT, kT [C, N] (head-dim on partitions); head pair 0 first so that
     attention starts while the rest of B still runs.
  C: v natural [N, 12*65], 65th col per head = 1.0 (softmax denominator).
  D: per head pair/n-chunk: scores S^T[m, n] as two K=64 matmuls in
     disjoint row groups (they overlap on the PE) into one [128, 1024]
     PSUM tile; one 1024-wide exp on ScalarE (scale fused) into bf16
     P tiles; AV matmul with M=65 (row 64 accumulates the denominator);
     raw out^T to SBUF; denominator rows collected into a [96, 128] tile.
     After every odd hp: 32-row reciprocal chunk + indicator-matmul
     broadcasts + DVE multiplies normalize the finished blocks.
  E: yT[o, n] = pwT.T @ outT with bias added on DVE.
Host transposes yT back to [N, C].
"""

import numpy as np

B, N, C, H, D = 8, 1024, 768, 12, 64
SCALE = D ** -0.5
NCORES = 8

CT = C // 128   # 6  c-tiles
HP = H // 2     # 6  head pairs (2 heads of 64 share a 128-partition tile)
NT = N // 512   # 2  n-chunks of 512
MT = N // 128   # 8  m-tiles (keys)
VW = 65         # v columns per head (64 data + 1 ones)

_CACHE = {}


def _build_nc():
    import concourse.bass as bass
    import concourse.mybir as mybir
    import concourse.tile as tile
    from concourse import bacc

    f32 = mybir.dt.float32
    bf16 = mybir.dt.bfloat16
    AF = mybir.ActivationFunctionType

    nc = bacc.Bacc(
        "TRN2",
        target_bir_lowering=False,
        debug=False,
        enable_asserts=True,
        num_devices=NCORES,
    )

    xT_d = nc.dram_tensor("xT", [C, N], bf16, kind="ExternalInput").ap()
    wq_d = nc.dram_tensor("wqT", [C, C], bf16, kind="ExternalInput").ap()
    wk_d = nc.dram_tensor("wkT", [C, C], bf16, kind="ExternalInput").ap()
    wv_d = nc.dram_tensor("wvT", [C, C], bf16, kind="ExternalInput").ap()
    pw_d = nc.dram_tensor("pwT", [C, C], bf16, kind="ExternalInput").ap()
    ones_d = nc.dram_tensor("ones2", [2, 128], bf16, kind="ExternalInput").ap()
    pb_d = nc.dram_tensor("pb", [128, CT], f32, kind="ExternalInput").ap()
    out_d = nc.dram_tensor("out", [C, N], f32, kind="ExternalOutput").ap()

    with tile.TileContext(nc) as tc:
        data = tc.alloc_tile_pool(name="data", bufs=1)
        ptp = tc.alloc_tile_pool(name="ptp", bufs=2)
        small = tc.alloc_tile_pool(name="small", bufs=4)
        psp = tc.alloc_tile_pool(name="psp", bufs=1, space="PSUM")

        pb_sb = data.tile([128, CT], f32)
        nc.sync.dma_start(pb_sb[:], pb_d[:])
        ones2 = data.tile([2, 128], bf16)
        nc.sync.dma_start(ones2[:], ones_d[:])

        xTs = data.tile([128, CT * N], bf16)
        wqs = data.tile([128, CT * C], bf16)
        wks = data.tile([128, CT * C], bf16)
        wvs = data.tile([128, CT * C], bf16)
        pws = data.tile([128, CT * C], bf16)
        for ci in range(CT):
            nc.sync.dma_start(xTs[:, ci * N:(ci + 1) * N],
                              xT_d[ci * 128:(ci + 1) * 128, :])
            nc.sync.dma_start(wqs[:, ci * C:(ci + 1) * C],
                              wq_d[ci * 128:(ci + 1) * 128, :])
            nc.sync.dma_start(wks[:, ci * C:(ci + 1) * C],
                              wk_d[ci * 128:(ci + 1) * 128, :])
            nc.sync.dma_start(wvs[:, ci * C:(ci + 1) * C],
                              wv_d[ci * 128:(ci + 1) * 128, :])
            nc.sync.dma_start(pws[:, ci * C:(ci + 1) * C],
                              pw_d[ci * 128:(ci + 1) * 128, :])

        qT = data.tile([128, HP * N], bf16)
        kT = data.tile([128, HP * N], bf16)
        va = data.tile([128, MT * H * VW], bf16)
        oT_raw = data.tile([128, HP * N], bf16)
        oT = data.tile([128, HP * N], bf16)
        den = data.tile([96, 128], f32)
        recip_all = data.tile([96, 128], bf16)

        # ones columns of v (softmax denominator trick)
        v3 = va[:].rearrange("p (x e) -> p x e", e=VW)
        nc.gpsimd.memset(v3[:, :, 64:65], 1.0)

        def b_block(hp):
            for dst, w in ((qT, wqs), (kT, wks)):
                for nn in range(NT):
                    ps = psp.tile([128, 512], f32, tag="acc", bufs=2,
                                  name="ps_qk")
                    for ci in range(CT):
                        nc.tensor.matmul(
                            ps[:],
                            w[:, ci * C + hp * 128: ci * C + hp * 128 + 128],
                            xTs[:, ci * N + nn * 512: ci * N + nn * 512 + 512],
                            start=(ci == 0), stop=(ci == CT - 1),
                        )
                    nc.vector.tensor_copy(
                        dst[:, hp * N + nn * 512: hp * N + nn * 512 + 512],
                        ps[:])

        # ---- B (head pair 0 only, so attention can start early) ----------
        b_block(0)

        # ---- C: v natural layout [tokens, head*65] -----------------------
        for mt in range(MT):
            for oc in range(2):
                ow = 512 if oc == 0 else 256
                nh = ow // 64
                ps = psp.tile([128, 512], f32, tag="acc", bufs=2, name="ps_v")
                for ci in range(CT):
                    nc.tensor.matmul(
                        ps[:, :ow],
                        xTs[:, ci * N + mt * 128: ci * N + mt * 128 + 128],
                        wvs[:, ci * C + oc * 512: ci * C + oc * 512 + ow],
                        start=(ci == 0), stop=(ci == CT - 1),
                    )
                dst3 = va[:, mt * H * VW:(mt + 1) * H * VW].rearrange(
                    "p (h e) -> p h e", e=VW)[:, oc * 8: oc * 8 + nh, 0:64]
                src3 = ps[:, :ow].rearrange("p (h d) -> p h d", d=64)
                nc.vector.tensor_copy(dst3, src3)

        # ---- rest of B ---------------------------------------------------
        for hp in range(1, HP):
            b_block(hp)

        # ---- softmax division (chunked so E can start early) -------------
        def do_division(chunk):
            rows = slice(32 * chunk, 32 * chunk + 32)
            with nc.allow_low_precision(reason="softmax denom in bf16"):
                nc.vector.reciprocal(recip_all[rows, :], den[rows, :])
            for j in range(4 * chunk, 4 * chunk + 4):
                hp_, nn_ = divmod(j, NT)
                recip2 = small.tile([2, 512], bf16, tag="recip2", name="recip2")
                for hi in range(2):
                    idx = 2 * j + hi
                    nc.sync.dma_start(recip2[hi:hi + 1, :],
                                      recip_all[4 * idx:4 * idx + 4, :])
                bc = psp.tile([128, 512], f32, tag="out", bufs=2, name="bc")
                nc.tensor.matmul(bc[:], ones2[:], recip2[:], start=True,
                                 stop=True)
                blk = slice(hp_ * N + nn_ * 512, hp_ * N + nn_ * 512 + 512)
                nc.vector.tensor_mul(oT[:, blk], oT_raw[:, blk], bc[:])

        # ---- D: attention per head pair ----------------------------------
        for hp in range(HP):
            for nn in range(NT):
                pt = ptp.tile([128, 2 * MT * 512], bf16, tag="pt", name="pt")
                pt3 = pt[:].rearrange("p (g x) -> p g x", g=2)
                for mt in range(MT):
                    st = psp.tile([128, 1024], f32, tag="st", bufs=2, name="st")
                    for hi in range(2):
                        lo = 64 * hi
                        nc.tensor.matmul(
                            st[:, hi * 512:(hi + 1) * 512],
                            kT[lo:lo + 64,
                               hp * N + mt * 128: hp * N + mt * 128 + 128],
                            qT[lo:lo + 64,
                               hp * N + nn * 512: hp * N + nn * 512 + 512],
                        )
                    nc.scalar.activation(
                        pt3[:, :, mt * 512:(mt + 1) * 512],
                        st[:].rearrange("p (g x) -> p g x", g=2),
                        AF.Exp, scale=SCALE)
                for hi in range(2):
                    h = 2 * hp + hi
                    lo = 64 * hi
                    av = psp.tile([128, 512], f32, tag="acc", bufs=2, name="av")
                    for mt in range(MT):
                        nc.tensor.matmul(
                            av[0:VW, :],
                            va[:, mt * H * VW + h * VW: mt * H * VW + h * VW + VW],
                            pt[:, hi * MT * 512 + mt * 512:
                               hi * MT * 512 + mt * 512 + 512],
                            start=(mt == 0), stop=(mt == MT - 1),
                        )
                    nc.vector.tensor_copy(
                        oT_raw[lo:lo + 64,
                               hp * N + nn * 512: hp * N + nn * 512 + 512],
                        av[0:64, :])
                    idx = (hp * NT + nn) * 2 + hi
                    dstage = small.tile([65, 512], f32, tag="dstage",
                                        name="dstage")
                    nc.vector.tensor_copy(dstage[64:65, :], av[64:65, :])
                    nc.sync.dma_start(den[4 * idx:4 * idx + 4, :],
                                      dstage[64:65, :])
            if hp % 2 == 1:
                do_division(hp // 2)

        # ---- E: output projection ----------------------------------------
        for ot in range(CT):
            for nn in range(NT):
                yp = psp.tile([128, 512], f32, tag="out", bufs=2, name="yp")
                for ci in range(CT):
                    nc.tensor.matmul(
                        yp[:],
                        pws[:, ci * C + ot * 128: ci * C + ot * 128 + 128],
                        oT[:, ci * N + nn * 512: ci * N + nn * 512 + 512],
                        start=(ci == 0), stop=(ci == CT - 1),
                    )
                ys = small.tile([128, 512], f32, tag="ys", name="ys")
                nc.vector.tensor_scalar_add(ys[:], yp[:], pb_sb[:, ot:ot + 1])
                nc.sync.dma_start(
                    out_d[ot * 128:(ot + 1) * 128, nn * 512:(nn + 1) * 512],
                    ys[:])

        psp.release()
        small.release()
        ptp.release()
        data.release()

    nc.compile()
    return nc


def _get_nc():
    if "nc" not in _CACHE:
        _CACHE["nc"] = _build_nc()
    return _CACHE["nc"]


def _prep_in_maps(x, qkv_w, proj_w, proj_b):
    import ml_dtypes

    bf16 = ml_dtypes.bfloat16
    x = np.asarray(x, dtype=np.float32)
    qkv_w = np.asarray(qkv_w, dtype=np.float32)
    proj_w = np.asarray(proj_w, dtype=np.float32)
    proj_b = np.asarray(proj_b, dtype=np.float32)

    wqT = np.ascontiguousarray(qkv_w[0:C].T).astype(bf16)     # [C(in), C(out)]
    wkT = np.ascontiguousarray(qkv_w[C:2 * C].T).astype(bf16)
    wvT = np.ascontiguousarray(qkv_w[2 * C:3 * C].T).astype(bf16)
    pwT = np.ascontiguousarray(proj_w.T).astype(bf16)
    pb = np.ascontiguousarray(proj_b.reshape(CT, 128).T)      # [128, CT] f32
    ones2 = np.zeros((2, 128), dtype=np.float32)
    ones2[0, 0:64] = 1.0
    ones2[1, 64:128] = 1.0
    ones2 = ones2.astype(bf16)

    in_maps = []
    for b in range(B):
        in_maps.append({
            "xT": np.ascontiguousarray(x[b].T).astype(bf16),
            "wqT": wqT, "wkT": wkT, "wvT": wvT, "pwT": pwT, "pb": pb,
            "ones2": ones2,
        })
    return in_maps


def _run(in_maps, **kwargs):
    from concourse.bass_utils import run_bass_kernel_spmd

    return run_bass_kernel_spmd(_get_nc(), in_maps,
                                core_ids=list(range(NCORES)), **kwargs)


def _gather(res):
    out = np.stack([res.results[b]["out"].T for b in range(B)], axis=0)
    return np.ascontiguousarray(out.astype(np.float32))


def kernel(x, qkv_w, proj_w, proj_b):
    return _gather(_run(_prep_in_maps(x, qkv_w, proj_w, proj_b)))


# revision 14
# speedup vs baseline: 1.2380x; 1.0439x over previous
"""Multi-head attention (B=8, N=1024, C=768, H=12, D=64) on 8 TRN2 NeuronCores.

Sharding: pure data parallelism — one batch element per core, no collectives.

Per-core dataflow (matmuls bf16, accumulation fp32 in PSUM):
  host prep: x[b].T, per-section qkv weights and proj weight transposed to
  [in, out], all bf16; proj bias fp32 [128, 6]; ones2 bf16 [2, 128]
  (indicator rows to broadcast softmax reciprocals across partitions).

  B: qT, kT [C, N] (head-dim on partitions); head pair 0 first so that
     attention starts while the rest of B still runs.
  C: v natural [N, 12*65], 65th col per head = 1.0 (softmax denominator).
  D: per head pair/n-chunk: scores S^T[m, n] as two K=64 matmuls in
     disjoint row groups (they overlap on the PE) into one [128, 1024]
     PSUM tile; one 1024-wide exp on ScalarE (scale fused) into bf16
     P tiles; AV matmul with M=65 (row 64 accumulates the denominator);
     raw out^T to SBUF; denominator rows collected into a [96, 128] tile.
     After every odd hp: 32-row reciprocal chunk + indicator-matmul
     broadcasts + DVE multiplies normalize the finished blocks.
  E: yT[o, n] = pwT.T @ outT with bias added on DVE.
Host transposes yT back to [N, C].
"""

import numpy as np

B, N, C, H, D = 8, 1024, 768, 12, 64
SCALE = D ** -0.5
NCORES = 8

CT = C // 128   # 6  c-tiles
HP = H // 2     # 6  head pairs (2 heads of 64 share a 128-partition tile)
NT = N // 512   # 2  n-chunks of 512
MT = N // 128   # 8  m-tiles (keys)
VW = 65         # v columns per head (64 data + 1 ones)

_CACHE = {}


def _build_nc():
    import concourse.bass as bass
    import concourse.mybir as mybir
    import concourse.tile as tile
    from concourse import bacc

    f32 = mybir.dt.float32
    bf16 = mybir.dt.bfloat16
    AF = mybir.ActivationFunctionType

    nc = bacc.Bacc(
        "TRN2",
        target_bir_lowering=False,
        debug=False,
        enable_asserts=True,
        num_devices=NCORES,
    )

    xT_d = nc.dram_tensor("xT", [C, N], bf16, kind="ExternalInput").ap()
    wq_d = nc.dram_tensor("wqT", [C, C], bf16, kind="ExternalInput").ap()
    wk_d = nc.dram_tensor("wkT", [C, C], bf16, kind="ExternalInput").ap()
    wv_d = nc.dram_tensor("wvT", [C, C], bf16, kind="ExternalInput").ap()
    pw_d = nc.dram_tensor("pwT", [C, C], bf16, kind="ExternalInput").ap()
    ones_d = nc.dram_tensor("ones2", [2, 128], bf16, kind="ExternalInput").ap()
    pb_d = nc.dram_tensor("pb", [128, CT], f32, kind="ExternalInput").ap()
    out_d = nc.dram_tensor("out", [C, N], f32, kind="ExternalOutput").ap()

    with tile.TileContext(nc) as tc:
        data = tc.alloc_tile_pool(name="data", bufs=1)
        ptp = tc.alloc_tile_pool(name="ptp", bufs=2)
        small = tc.alloc_tile_pool(name="small", bufs=4)
        psp = tc.alloc_tile_pool(name="psp", bufs=1, space="PSUM")

        pb_sb = data.tile([128, CT], f32)
        nc.sync.dma_start(pb_sb[:], pb_d[:])
        ones2 = data.tile([2, 128], bf16)
        nc.sync.dma_start(ones2[:], ones_d[:])

        xTs = data.tile([128, CT * N], bf16)
        wqs = data.tile([128, CT * C], bf16)
        wks = data.tile([128, CT * C], bf16)
        wvs = data.tile([128, CT * C], bf16)
        pws = data.tile([128, CT * C], bf16)
        for ci in range(CT):
            nc.sync.dma_start(xTs[:, ci * N:(ci + 1) * N],
                              xT_d[ci * 128:(ci + 1) * 128, :])
            nc.sync.dma_start(wqs[:, ci * C:(ci + 1) * C],
                              wq_d[ci * 128:(ci + 1) * 128, :])
            nc.sync.dma_start(wks[:, ci * C:(ci + 1) * C],
                              wk_d[ci * 128:(ci + 1) * 128, :])
        for ci in range(CT):
            nc.sync.dma_start(wvs[:, ci * C:(ci + 1) * C],
                              wv_d[ci * 128:(ci + 1) * 128, :])
        for ci in range(CT):
            nc.sync.dma_start(pws[:, ci * C:(ci + 1) * C],
                              pw_d[ci * 128:(ci + 1) * 128, :])

        qT = data.tile([128, HP * N], bf16)
        kT = data.tile([128, HP * N], bf16)
        va = data.tile([128, MT * H * VW], bf16)
        oT_raw = data.tile([128, HP * N], bf16)
        oT = data.tile([128, HP * N], bf16)
        den = data.tile([128, 128], f32)
        recip_all = data.tile([128, 128], bf16)
        nc.gpsimd.memset(den[:], 1.0)
        DEN_BASE = {0: 0, 1: 16, 2: 32, 3: 48, 4: 64, 5: 96}

        def den_row(hp, nn, hi):
            return DEN_BASE[hp] + (nn * 2 + hi) * 4

        # ones columns of v (softmax denominator trick)
        v3 = va[:].rearrange("p (x e) -> p x e", e=VW)
        nc.gpsimd.memset(v3[:, :, 64:65], 1.0)

        def b_block(hp):
            for dst, w in ((qT, wqs), (kT, wks)):
                for nn in range(NT):
                    ps = psp.tile([128, 512], f32, tag="acc", bufs=2,
                                  name="ps_qk")
                    for ci in range(CT):
                        nc.tensor.matmul(
                            ps[:],
                            w[:, ci * C + hp * 128: ci * C + hp * 128 + 128],
                            xTs[:, ci * N + nn * 512: ci * N + nn * 512 + 512],
                            start=(ci == 0), stop=(ci == CT - 1),
                        )
                    nc.vector.tensor_copy(
                        dst[:, hp * N + nn * 512: hp * N + nn * 512 + 512],
                        ps[:])

        # ---- B (head pair 0 only, so attention can start early) ----------
        b_block(0)

        # ---- C: v natural layout [tokens, head*65] -----------------------
        for mt in range(MT):
            for oc in range(2):
                ow = 512 if oc == 0 else 256
                nh = ow // 64
                ps = psp.tile([128, 512], f32, tag="acc", bufs=2, name="ps_v")
                for ci in range(CT):
                    nc.tensor.matmul(
                        ps[:, :ow],
                        xTs[:, ci * N + mt * 128: ci * N + mt * 128 + 128],
                        wvs[:, ci * C + oc * 512: ci * C + oc * 512 + ow],
                        start=(ci == 0), stop=(ci == CT - 1),
                    )
                dst3 = va[:, mt * H * VW:(mt + 1) * H * VW].rearrange(
                    "p (h e) -> p h e", e=VW)[:, oc * 8: oc * 8 + nh, 0:64]
                src3 = ps[:, :ow].rearrange("p (h d) -> p h d", d=64)
                nc.vector.tensor_copy(dst3, src3)

        # ---- rest of B ---------------------------------------------------
        for hp in range(1, HP):
            b_block(hp)

        # ---- softmax division (chunked so E can start early) -------------
        def do_division(chunk):
            rows = slice(32 * chunk, 32 * chunk + 32)
            with nc.allow_low_precision(reason="softmax denom in bf16"):
                nc.vector.reciprocal(recip_all[rows, :], den[rows, :])
            js = {0: (0, 1, 2, 3), 1: (4, 5, 6, 7), 2: (8, 9), 3: (10, 11)}[chunk]
            for j in js:
                hp_, nn_ = divmod(j, NT)
                recip2 = small.tile([2, 512], bf16, tag="recip2", name="recip2")
                for hi in range(2):
                    rb = den_row(hp_, nn_, hi)
                    nc.sync.dma_start(recip2[hi:hi + 1, :],
                                      recip_all[rb:rb + 4, :])
                bc = psp.tile([128, 512], f32, tag="out", bufs=2, name="bc")
                nc.tensor.matmul(bc[:], ones2[:], recip2[:], start=True,
                                 stop=True)
                blk = slice(hp_ * N + nn_ * 512, hp_ * N + nn_ * 512 + 512)
                nc.vector.tensor_mul(oT[:, blk], oT_raw[:, blk], bc[:])

        # ---- D: attention per head pair ----------------------------------
        for hp in range(HP):
            for nn in range(NT):
                pt = ptp.tile([128, 2 * MT * 512], bf16, tag="pt", name="pt")
                pt3 = pt[:].rearrange("p (g x) -> p g x", g=2)
                for mt in range(MT):
                    st = psp.tile([128, 1024], f32, tag="st", bufs=2, name="st")
                    for hi in range(2):
                        lo = 64 * hi
                        nc.tensor.matmul(
                            st[:, hi * 512:(hi + 1) * 512],
                            kT[lo:lo + 64,
                               hp * N + mt * 128: hp * N + mt * 128 + 128],
                            qT[lo:lo + 64,
                               hp * N + nn * 512: hp * N + nn * 512 + 512],
                        )
                    nc.scalar.activation(
                        pt3[:, :, mt * 512:(mt + 1) * 512],
                        st[:].rearrange("p (g x) -> p g x", g=2),
                        AF.Exp, scale=SCALE)
                for hi in range(2):
                    h = 2 * hp + hi
                    lo = 64 * hi
                    av = psp.tile([128, 512], f32, tag="acc", bufs=2, name="av")
                    for mt in range(MT):
                        nc.tensor.matmul(
                            av[0:VW, :],
                            va[:, mt * H * VW + h * VW: mt * H * VW + h * VW + VW],
                            pt[:, hi * MT * 512 + mt * 512:
                               hi * MT * 512 + mt * 512 + 512],
                            start=(mt == 0), stop=(mt == MT - 1),
                        )
                    dstage = small.tile([65, 512], f32, tag="dstage",
                                        name="dstage")
                    nc.vector.tensor_copy(dstage[64:65, :], av[64:65, :])
                    rb = den_row(hp, nn, hi)
                    nc.sync.dma_start(den[rb:rb + 4, :], dstage[64:65, :])
                    nc.vector.tensor_copy(
                        oT_raw[lo:lo + 64,
                               hp * N + nn * 512: hp * N + nn * 512 + 512],
                        av[0:64, :])
            if hp in (1, 3, 4, 5):
                do_division({1: 0, 3: 1, 4: 2, 5: 3}[hp])

        # ---- E: output projection ----------------------------------------
        for ot in range(CT):
            for nn in range(NT):
                ytag = "st" if (ot * NT + nn) % 2 else "out"
                yp = psp.tile([128, 512], f32, tag=ytag, bufs=2, name="yp")
                for ci in range(CT):
                    nc.tensor.matmul(
                        yp[:],
                        pws[:, ci * C + ot * 128: ci * C + ot * 128 + 128],
                        oT[:, ci * N + nn * 512: ci * N + nn * 512 + 512],
                        start=(ci == 0), stop=(ci == CT - 1),
                    )
                ys = small.tile([128, 512], f32, tag="ys", name="ys")
                nc.vector.tensor_scalar_add(ys[:], yp[:], pb_sb[:, ot:ot + 1])
                nc.sync.dma_start(
                    out_d[ot * 128:(ot + 1) * 128, nn * 512:(nn + 1) * 512],
                    ys[:])

        psp.release()
        small.release()
        ptp.release()
        data.release()

    nc.compile()
    return nc


def _get_nc():
    if "nc" not in _CACHE:
        _CACHE["nc"] = _build_nc()
    return _CACHE["nc"]


def _prep_in_maps(x, qkv_w, proj_w, proj_b):
    import ml_dtypes

    bf16 = ml_dtypes.bfloat16
    x = np.asarray(x, dtype=np.float32)
    qkv_w = np.asarray(qkv_w, dtype=np.float32)
    proj_w = np.asarray(proj_w, dtype=np.float32)
    proj_b = np.asarray(proj_b, dtype=np.float32)

    wqT = np.ascontiguousarray(qkv_w[0:C].T).astype(bf16)     # [C(in), C(out)]
    wkT = np.ascontiguousarray(qkv_w[C:2 * C].T).astype(bf16)
    wvT = np.ascontiguousarray(qkv_w[2 * C:3 * C].T).astype(bf16)
    pwT = np.ascontiguousarray(proj_w.T).astype(bf16)
    pb = np.ascontiguousarray(proj_b.reshape(CT, 128).T)      # [128, CT] f32
    ones2 = np.zeros((2, 128), dtype=np.float32)
    ones2[0, 0:64] = 1.0
    ones2[1, 64:128] = 1.0
    ones2 = ones2.astype(bf16)

    in_maps = []
    for b in range(B):
        in_maps.append({
            "xT": np.ascontiguousarray(x[b].T).astype(bf16),
            "wqT": wqT, "wkT": wkT, "wvT": wvT, "pwT": pwT, "pb": pb,
            "ones2": ones2,
        })
    return in_maps


def _run(in_maps, **kwargs):
    from concourse.bass_utils import run_bass_kernel_spmd

    return run_bass_kernel_spmd(_get_nc(), in_maps,
                                core_ids=list(range(NCORES)), **kwargs)


def _gather(res):
    out = np.stack([res.results[b]["out"].T for b in range(B)], axis=0)
    return np.ascontiguousarray(out.astype(np.float32))


def kernel(x, qkv_w, proj_w, proj_b):
    return _gather(_run(_prep_in_maps(x, qkv_w, proj_w, proj_b)))


# revision 15
# speedup vs baseline: 1.2724x; 1.0278x over previous
"""Multi-head attention (B=8, N=1024, C=768, H=12, D=64) on 8 TRN2 NeuronCores.

Sharding: pure data parallelism — one batch element per core, no collectives.

Per-core dataflow (matmuls bf16, accumulation fp32 in PSUM):
  host prep: x[b].T, per-section qkv weights and proj weight transposed to
  [in, out], all bf16; proj bias fp32 [128, 6]; ones2 bf16 [2, 128]
  (indicator rows to broadcast softmax reciprocals across partitions).

  B: qT, kT [C, N] (head-dim on partitions); head pair 0 first so that
     attention starts while the rest of B still runs.
  C: v natural [N, 12*65], 65th col per head = 1.0 (softmax denominator).
  D: per head pair/n-chunk: scores S^T[m, n] as two K=64 matmuls in
     disjoint row groups (they overlap on the PE) into one [128, 1024]
     PSUM tile; one 1024-wide exp on ScalarE (scale fused) into bf16
     P tiles; AV matmul with M=65 (row 64 accumulates the denominator);
     raw out^T to SBUF; denominator rows collected into a [96, 128] tile.
     After every odd hp: 32-row reciprocal chunk + indicator-matmul
     broadcasts + DVE multiplies normalize the finished blocks.
  E: yT[o, n] = pwT.T @ outT with bias added on DVE.
Host transposes yT back to [N, C].
"""

import numpy as np

B, N, C, H, D = 8, 1024, 768, 12, 64
SCALE = D ** -0.5
NCORES = 8

CT = C // 128   # 6  c-tiles
HP = H // 2     # 6  head pairs (2 heads of 64 share a 128-partition tile)
NT = N // 512   # 2  n-chunks of 512
MT = N // 128   # 8  m-tiles (keys)
VW = 80         # v cols/head (64 data + ones + pad; 16B-aligned strides for fp8 DoubleRow)

_CACHE = {}


def _build_nc():
    import concourse.bass as bass
    import concourse.mybir as mybir
    import concourse.tile as tile
    from concourse import bacc

    f32 = mybir.dt.float32
    bf16 = mybir.dt.bfloat16
    fp8 = mybir.dt.float8e4
    AF = mybir.ActivationFunctionType
    PM = mybir.MatmulPerfMode

    nc = bacc.Bacc(
        "TRN2",
        target_bir_lowering=False,
        debug=False,
        enable_asserts=True,
        num_devices=NCORES,
    )

    xT_d = nc.dram_tensor("xT", [C, N], bf16, kind="ExternalInput").ap()
    wq_d = nc.dram_tensor("wqT", [C, C], bf16, kind="ExternalInput").ap()
    wk_d = nc.dram_tensor("wkT", [C, C], bf16, kind="ExternalInput").ap()
    wv_d = nc.dram_tensor("wvT", [C, C], bf16, kind="ExternalInput").ap()
    pw_d = nc.dram_tensor("pwT", [C, C], bf16, kind="ExternalInput").ap()
    ones_d = nc.dram_tensor("ones2", [2, 128], bf16, kind="ExternalInput").ap()
    pb_d = nc.dram_tensor("pb", [128, CT], f32, kind="ExternalInput").ap()
    out_d = nc.dram_tensor("out", [C, N], f32, kind="ExternalOutput").ap()

    with tile.TileContext(nc) as tc:
        data = tc.alloc_tile_pool(name="data", bufs=1)
        ptp = tc.alloc_tile_pool(name="ptp", bufs=2)
        small = tc.alloc_tile_pool(name="small", bufs=4)
        psp = tc.alloc_tile_pool(name="psp", bufs=1, space="PSUM")

        pb_sb = data.tile([128, CT], f32)
        nc.sync.dma_start(pb_sb[:], pb_d[:])
        ones2 = data.tile([2, 128], bf16)
        nc.sync.dma_start(ones2[:], ones_d[:])

        xTs = data.tile([128, CT * N], bf16)
        wqs = data.tile([128, CT * C], bf16)
        wks = data.tile([128, CT * C], bf16)
        wvs = data.tile([128, CT * C], bf16)
        pws = data.tile([128, CT * C], bf16)
        for ci in range(CT):
            nc.sync.dma_start(xTs[:, ci * N:(ci + 1) * N],
                              xT_d[ci * 128:(ci + 1) * 128, :])
            nc.sync.dma_start(wqs[:, ci * C:(ci + 1) * C],
                              wq_d[ci * 128:(ci + 1) * 128, :])
            nc.sync.dma_start(wks[:, ci * C:(ci + 1) * C],
                              wk_d[ci * 128:(ci + 1) * 128, :])
        for ci in range(CT):
            nc.sync.dma_start(wvs[:, ci * C:(ci + 1) * C],
                              wv_d[ci * 128:(ci + 1) * 128, :])
        for ci in range(CT):
            nc.sync.dma_start(pws[:, ci * C:(ci + 1) * C],
                              pw_d[ci * 128:(ci + 1) * 128, :])

        qT = data.tile([128, HP * N], bf16)
        kT = data.tile([128, HP * N], bf16)
        va = data.tile([128, MT * H * VW], fp8)
        oT_raw = data.tile([128, HP * N], bf16)
        oT = data.tile([128, HP * N], bf16)
        den = data.tile([128, 128], f32)
        recip_all = data.tile([128, 128], bf16)
        nc.gpsimd.memset(den[:], 1.0)
        DEN_BASE = {0: 0, 1: 16, 2: 32, 3: 48, 4: 64, 5: 96}

        def den_row(hp, nn, hi):
            return DEN_BASE[hp] + (nn * 2 + hi) * 4

        # ones columns of v (softmax denominator trick)
        v3 = va[:].rearrange("p (x e) -> p x e", e=VW)
        nc.gpsimd.memset(v3[:, :, 64:65], 1.0)

        def b_block(hp):
            for dst, w in ((qT, wqs), (kT, wks)):
                for nn in range(NT):
                    ps = psp.tile([128, 512], f32, tag="acc", bufs=2,
                                  name="ps_qk")
                    for ci in range(CT):
                        nc.tensor.matmul(
                            ps[:],
                            w[:, ci * C + hp * 128: ci * C + hp * 128 + 128],
                            xTs[:, ci * N + nn * 512: ci * N + nn * 512 + 512],
                            start=(ci == 0), stop=(ci == CT - 1),
                        )
                    nc.vector.tensor_copy(
                        dst[:, hp * N + nn * 512: hp * N + nn * 512 + 512],
                        ps[:])

        # ---- B (head pair 0 only, so attention can start early) ----------
        b_block(0)

        # ---- C: v natural layout [tokens, head*65] -----------------------
        for mt in range(MT):
            for oc in range(2):
                ow = 512 if oc == 0 else 256
                nh = ow // 64
                ps = psp.tile([128, 512], f32, tag="acc", bufs=2, name="ps_v")
                for ci in range(CT):
                    nc.tensor.matmul(
                        ps[:, :ow],
                        xTs[:, ci * N + mt * 128: ci * N + mt * 128 + 128],
                        wvs[:, ci * C + oc * 512: ci * C + oc * 512 + ow],
                        start=(ci == 0), stop=(ci == CT - 1),
                    )
                dst3 = va[:, mt * H * VW:(mt + 1) * H * VW].rearrange(
                    "p (h e) -> p h e", e=VW)[:, oc * 8: oc * 8 + nh, 0:64]
                src3 = ps[:, :ow].rearrange("p (h d) -> p h d", d=64)
                nc.vector.tensor_copy(dst3, src3)

        # ---- rest of B ---------------------------------------------------
        for hp in range(1, HP):
            b_block(hp)

        # ---- softmax division (chunked so E can start early) -------------
        def do_division(chunk):
            rows = slice(32 * chunk, 32 * chunk + 32)
            with nc.allow_low_precision(reason="softmax denom in bf16"):
                nc.vector.reciprocal(recip_all[rows, :], den[rows, :])
            js = {0: (0, 1, 2, 3), 1: (4, 5, 6, 7), 2: (8, 9), 3: (10, 11)}[chunk]
            for j in js:
                hp_, nn_ = divmod(j, NT)
                recip2 = small.tile([2, 512], bf16, tag="recip2", name="recip2")
                for hi in range(2):
                    rb = den_row(hp_, nn_, hi)
                    nc.sync.dma_start(recip2[hi:hi + 1, :],
                                      recip_all[rb:rb + 4, :])
                bc = psp.tile([128, 512], f32, tag="out", bufs=2, name="bc")
                nc.tensor.matmul(bc[:], ones2[:], recip2[:], start=True,
                                 stop=True)
                blk = slice(hp_ * N + nn_ * 512, hp_ * N + nn_ * 512 + 512)
                nc.vector.tensor_mul(oT[:, blk], oT_raw[:, blk], bc[:])

        # ---- D: attention per head pair ----------------------------------
        for hp in range(HP):
            for nn in range(NT):
                pt = ptp.tile([128, 2 * MT * 512], fp8, tag="pt", name="pt")
                pt3 = pt[:].rearrange("p (g x) -> p g x", g=2)
                for mt in range(MT):
                    st = psp.tile([128, 1024], f32, tag="st", bufs=2, name="st")
                    for hi in range(2):
                        lo = 64 * hi
                        nc.tensor.matmul(
                            st[:, hi * 512:(hi + 1) * 512],
                            kT[lo:lo + 64,
                               hp * N + mt * 128: hp * N + mt * 128 + 128],
                            qT[lo:lo + 64,
                               hp * N + nn * 512: hp * N + nn * 512 + 512],
                        )
                    nc.scalar.activation(
                        pt3[:, :, mt * 512:(mt + 1) * 512],
                        st[:].rearrange("p (g x) -> p g x", g=2),
                        AF.Exp, scale=SCALE)
                va4 = va[:].rearrange("p (mt h e) -> p mt h e", h=H, e=VW)
                for hi in range(2):
                    h = 2 * hp + hi
                    lo = 64 * hi
                    pt3 = pt[:, hi * MT * 512:(hi + 1) * MT * 512].rearrange(
                        "p (mt x) -> p mt x", mt=MT)
                    av = psp.tile([128, 512], f32, tag="acc", bufs=2, name="av")
                    for w in range(MT // 2):
                        nc.tensor.matmul(
                            av[0:VW, :],
                            va4[:, 2 * w:2 * w + 2, h, :],
                            pt3[:, 2 * w:2 * w + 2, :],
                            start=(w == 0), stop=(w == MT // 2 - 1),
                            perf_mode=PM.DoubleRow,
                        )
                    dstage = small.tile([65, 512], f32, tag="dstage",
                                        name="dstage")
                    nc.vector.tensor_copy(dstage[64:65, :], av[64:65, :])
                    rb = den_row(hp, nn, hi)
                    nc.sync.dma_start(den[rb:rb + 4, :], dstage[64:65, :])
                    nc.vector.tensor_copy(
                        oT_raw[lo:lo + 64,
                               hp * N + nn * 512: hp * N + nn * 512 + 512],
                        av[0:64, :])
            if hp in (1, 3, 4, 5):
                do_division({1: 0, 3: 1, 4: 2, 5: 3}[hp])

        # ---- E: output projection ----------------------------------------
        for ot in range(CT):
            for nn in range(NT):
                ytag = "st" if (ot * NT + nn) % 2 else "out"
                yp = psp.tile([128, 512], f32, tag=ytag, bufs=2, name="yp")
                for ci in range(CT):
                    nc.tensor.matmul(
                        yp[:],
                        pws[:, ci * C + ot * 128: ci * C + ot * 128 + 128],
                        oT[:, ci * N + nn * 512: ci * N + nn * 512 + 512],
                        start=(ci == 0), stop=(ci == CT - 1),
                    )
                ys = small.tile([128, 512], f32, tag="ys", name="ys")
                nc.vector.tensor_scalar_add(ys[:], yp[:], pb_sb[:, ot:ot + 1])
                nc.sync.dma_start(
                    out_d[ot * 128:(ot + 1) * 128, nn * 512:(nn + 1) * 512],
                    ys[:])

        psp.release()
        small.release()
        ptp.release()
        data.release()

    nc.compile()
    return nc


def _get_nc():
    if "nc" not in _CACHE:
        _CACHE["nc"] = _build_nc()
    return _CACHE["nc"]


def _prep_in_maps(x, qkv_w, proj_w, proj_b):
    import ml_dtypes

    bf16 = ml_dtypes.bfloat16
    x = np.asarray(x, dtype=np.float32)
    qkv_w = np.asarray(qkv_w, dtype=np.float32)
    proj_w = np.asarray(proj_w, dtype=np.float32)
    proj_b = np.asarray(proj_b, dtype=np.float32)

    wqT = np.ascontiguousarray(qkv_w[0:C].T).astype(bf16)     # [C(in), C(out)]
    wkT = np.ascontiguousarray(qkv_w[C:2 * C].T).astype(bf16)
    wvT = np.ascontiguousarray(qkv_w[2 * C:3 * C].T).astype(bf16)
    pwT = np.ascontiguousarray(proj_w.T).astype(bf16)
    pb = np.ascontiguousarray(proj_b.reshape(CT, 128).T)      # [128, CT] f32
    ones2 = np.zeros((2, 128), dtype=np.float32)
    ones2[0, 0:64] = 1.0
    ones2[1, 64:128] = 1.0
    ones2 = ones2.astype(bf16)

    in_maps = []
    for b in range(B):
        in_maps.append({
            "xT": np.ascontiguousarray(x[b].T).astype(bf16),
            "wqT": wqT, "wkT": wkT, "wvT": wvT, "pwT": pwT, "pb": pb,
            "ones2": ones2,
        })
    return in_maps


def _run(in_maps, **kwargs):
    from concourse.bass_utils import run_bass_kernel_spmd

    return run_bass_kernel_spmd(_get_nc(), in_maps,
                                core_ids=list(range(NCORES)), **kwargs)


def _gather(res):
    out = np.stack([res.results[b]["out"].T for b in range(B)], axis=0)
    return np.ascontiguousarray(out.astype(np.float32))


def kernel(x, qkv_w, proj_w, proj_b):
    return _gather(_run(_prep_in_maps(x, qkv_w, proj_w, proj_b)))
